# revision 19
# baseline (speedup 1.0000x reference)
"""CGConv GNN encoder (4 message-passing layers + softmax) on 8 Trainium2
NeuronCores via Bass/Tile.

Edge-parallel sharding: edges sorted by dst; dst nodes split into 8
contiguous ranges with balanced edge counts (1 per core). Per core, nodes
pack into blocks (<= NW nodes, <= TPB*128 edges); every core runs the same
program (SPMD), all structure padded to uniform shape.

Per layer:
  proj:  hT = DMAtranspose(h); per block: [acat|bcat] = hT_blk^T @ Wds
         acat -> SBUF rhs table rows 0:NW; bcat -> DRAM, AllGather.
  edges: per 128-edge tile:
         onehot_en[e,n] = (iota == dst_rel[e])         (DVE)
         onehot_ne     = PE-transpose(onehot_en)       (PE)
         lhs = [onehot_ne(rows 0:92); eaT(rows 92:125)]
         pre = lhs^T @ [acat_blk(0:92); We+bias(92:125)]  (PE->PSUM)
         pre += bcat[src]   (indirect gather + DVE add)
         msg = sigmoid(pre[:,:64])*softplus(pre[:,64:]) (ACT,DVE)
         agg_blk += onehot_en^T @ msg                   (PE accumulate)
  evac:  h' = relu(h + agg) (layers 1-3) / softmax(h + agg) (layer 4)
"""
import numpy as np
import ml_dtypes

import concourse.bass as bass
import concourse.bacc as bacc
import concourse.tile as tile
from concourse import mybir
from concourse.bass_utils import run_bass_kernel_spmd

BF16 = np.dtype(ml_dtypes.bfloat16)
F32 = mybir.dt.float32
BF = mybir.dt.bfloat16
I32 = mybir.dt.int32

P = 128
NC = 8
F = 64
DE = 32
FS = 128
NW = 64          # onehot/acat rows 0:64 (32-aligned partition bases req'd)
ROW_EA = 64      # eaT/We rows 64:97
KE = DE + 1      # 33
KTOT = ROW_EA + KE  # 97


def _pack_blocks(node_lo, node_hi, counts, tpb):
    blocks = []
    n = node_lo
    while n < node_hi:
        nodes = 0
        edges = 0
        start = n
        while n < node_hi and nodes < NW and edges + counts[n] <= tpb * P:
            edges += int(counts[n])
            nodes += 1
            n += 1
        if nodes == 0:
            raise RuntimeError("node degree exceeds block capacity")
        blocks.append((start, n))
    return blocks


def kernel(x, edge_index, edge_attr,
           Wf1, bf1, Ws1, bs1,
           Wf2, bf2, Ws2, bs2,
           Wf3, bf3, Ws3, bs3):
    x = np.asarray(x, np.float32)
    edge_index = np.asarray(edge_index)
    edge_attr = np.asarray(edge_attr, np.float32)
    Wsets = [(np.asarray(Wf1, np.float32), np.asarray(bf1, np.float32),
              np.asarray(Ws1, np.float32), np.asarray(bs1, np.float32)),
             (np.asarray(Wf2, np.float32), np.asarray(bf2, np.float32),
              np.asarray(Ws2, np.float32), np.asarray(bs2, np.float32)),
             (np.asarray(Wf3, np.float32), np.asarray(bf3, np.float32),
              np.asarray(Ws3, np.float32), np.asarray(bs3, np.float32))]

    n_nodes = x.shape[0]
    src_g = edge_index[0].astype(np.int64)
    dst_g = edge_index[1].astype(np.int64)
    order = np.argsort(dst_g, kind="stable")
    dst_s = dst_g[order]
    src_s = src_g[order]
    ea_s = edge_attr[order]

    counts = np.bincount(dst_s, minlength=n_nodes)
    cum = np.cumsum(counts)
    bounds = [0]
    for c in range(1, NC):
        bounds.append(int(np.searchsorted(cum, c * len(dst_s) / NC)))
    bounds.append(n_nodes)
    edge_ofs = np.concatenate([[0], cum]).astype(np.int64)

    tpb = 9
    per_core_blocks = [
        _pack_blocks(bounds[c], bounds[c + 1], counts, tpb) for c in range(NC)]
    B = max(len(b) for b in per_core_blocks)
    ECAP = B * tpb * P
    NROWS = B * P

    node_core = np.zeros(n_nodes, np.int32)
    node_lid = np.zeros(n_nodes, np.int32)
    for c in range(NC):
        for k, (a, b) in enumerate(per_core_blocks[c]):
            ids = np.arange(a, b)
            node_core[ids] = c
            node_lid[ids] = k * P + (ids - a)
    node_pgid = node_core.astype(np.int64) * NROWS + node_lid

    src_idx = np.zeros((NC, B, tpb, P), np.int32)
    dstrel = np.full((NC, B, tpb, P), -1.0, np.float32)
    eaT_sh = np.zeros((NC, KE, ECAP), np.float32)

    for c in range(NC):
        for k, (a, b) in enumerate(per_core_blocks[c]):
            e0, e1 = int(edge_ofs[a]), int(edge_ofs[b])
            ne = e1 - e0
            sl = np.arange(ne)
            t_i, p_i = sl // P, sl % P
            src_idx[c, k, t_i, p_i] = node_pgid[src_s[e0:e1]]
            dstrel[c, k, t_i, p_i] = node_lid[dst_s[e0:e1]] - k * P
            cols = (k * tpb + t_i) * P + p_i
            eaT_sh[c, 0:DE, cols] = ea_s[e0:e1]
            eaT_sh[c, DE, cols] = 1.0

    x_own = np.zeros((NC, NROWS, F), np.float32)
    for c in range(NC):
        for k, (a, b) in enumerate(per_core_blocks[c]):
            x_own[c, k * P:k * P + (b - a)] = x[a:b]

    w_in = {}
    for li, (Wf, bfv, Wsm, bsv) in enumerate(Wsets):
        wd = np.concatenate([Wf[0:F], Wsm[0:F]], axis=1)
        ws = np.concatenate([Wf[F:2 * F], Wsm[F:2 * F]], axis=1)
        we = np.concatenate([Wf[2 * F:], Wsm[2 * F:]], axis=1)
        bias = np.concatenate([bfv, bsv])[None, :]
        we_sh = np.zeros((P, FS), np.float32)
        we_sh[ROW_EA:ROW_EA + DE] = we
        we_sh[ROW_EA + DE] = bias
        w_in[f"wds{li}"] = np.concatenate([wd, ws], axis=1)
        w_in[f"we{li}"] = we_sh

    iota = np.broadcast_to(np.arange(P, dtype=np.float32)[None, :], (P, P))
    ident = np.eye(P, dtype=np.float32)

    nc = _build(B, tpb)

    in_maps = []
    for c in range(NC):
        m = dict(
            x_own=np.ascontiguousarray(x_own[c]),
            eaT_sh=np.ascontiguousarray(eaT_sh[c]),
            src_idx=np.ascontiguousarray(src_idx[c].transpose(0, 2, 1)),
            dst_rel=np.ascontiguousarray(dstrel[c].transpose(0, 2, 1)),
            iota=np.ascontiguousarray(np.array(iota)),
            ident=np.ascontiguousarray(ident),
            dst_row=np.ascontiguousarray(dstrel[c].reshape(B, 1, tpb * P)),
            ones_col=np.ones((1, P), np.float32),
            iota32=np.ascontiguousarray(np.arange(P, dtype=np.float32)[:, None]),
        )
        m.update({k: np.ascontiguousarray(v) for k, v in w_in.items()})
        in_maps.append(m)

    import os, time as _time
    res = run_bass_kernel_spmd(nc, in_maps, core_ids=list(range(NC)))
    if os.environ.get("PROFILE_RUNS"):
        for _ in range(int(os.environ["PROFILE_RUNS"])):
            t0 = _time.time()
            res = run_bass_kernel_spmd(nc, in_maps, core_ids=list(range(NC)))
            print("rerun wall (upload+exec+download): %.3fs" % (_time.time() - t0))
    if getattr(res, "exec_time_ns", None):
        print("HW exec time:", res.exec_time_ns, "ns")
    if getattr(res, "profile_json", None):
        import json
        with open("/root/problem/profile_last.json", "w") as f:
            json.dump(res.profile_json, f)

    out = np.zeros((n_nodes, F), np.float32)
    for c in range(NC):
        h4 = res.results[c]["out"]
        for k, (a, b) in enumerate(per_core_blocks[c]):
            out[a:b] = h4[k * P:k * P + (b - a)]
    return out.astype(np.float32)


def _build(B, tpb):
    ECAP = B * tpb * P
    NROWS = B * P
    nc = bacc.Bacc("TRN2", target_bir_lowering=False, debug=False,
                   num_devices=NC)

    t_x = nc.dram_tensor("x_own", [NROWS, F], F32, kind="ExternalInput")
    t_eaT = nc.dram_tensor("eaT_sh", [KE, ECAP], F32, kind="ExternalInput")
    t_srci = nc.dram_tensor("src_idx", [B, P, tpb], I32, kind="ExternalInput")
    t_drel = nc.dram_tensor("dst_rel", [B, P, tpb], F32, kind="ExternalInput")
    t_iota = nc.dram_tensor("iota", [P, P], F32, kind="ExternalInput")
    t_ident = nc.dram_tensor("ident", [P, P], F32, kind="ExternalInput")
    t_drow = nc.dram_tensor("dst_row", [B, 1, tpb * P], F32, kind="ExternalInput")
    t_ones = nc.dram_tensor("ones_col", [1, P], F32, kind="ExternalInput")
    t_iota32 = nc.dram_tensor("iota32", [P, 1], F32, kind="ExternalInput")
    t_wds = [nc.dram_tensor(f"wds{li}", [F, 2 * FS], F32, kind="ExternalInput")
             for li in range(3)]
    t_we = [nc.dram_tensor(f"we{li}", [P, FS], F32, kind="ExternalInput")
            for li in range(3)]
    t_out = nc.dram_tensor("out", [NROWS, F], F32, kind="ExternalOutput")

    h_a = nc.dram_tensor("h_a", [NROWS, F], F32, kind="Internal")
    h_b = nc.dram_tensor("h_b", [NROWS, F], F32, kind="Internal")
    bc_own = nc.dram_tensor("bc_own", [NROWS, FS], F32, kind="Internal")
    bc_full = nc.dram_tensor("bc_full", [NC * NROWS, FS], F32, kind="Internal")

    with tile.TileContext(nc) as tc:
        with tc.tile_pool(name="const", bufs=1) as cpool, \
             tc.tile_pool(name="persist", bufs=1) as pers, \
             tc.tile_pool(name="sb", bufs=4) as sb, \
             tc.tile_pool(name="blk", bufs=2) as blk, \
             tc.tile_pool(name="ps", bufs=3, space="PSUM") as ps, \
             tc.tile_pool(name="psagg", bufs=2, space="PSUM") as psagg:

            iota_t = cpool.tile([P, P], F32)
            nc.sync.dma_start(iota_t[:], t_iota.ap()[:, :])
            ident_t = cpool.tile([P, P], F32)
            nc.sync.dma_start(ident_t[:], t_ident.ap()[:, :])
            ones_t = cpool.tile([1, P], F32)
            nc.sync.dma_start(ones_t[:], t_ones.ap()[:, :])
            iota32_t = cpool.tile([P, 1], F32)
            nc.sync.dma_start(iota32_t[:], t_iota32.ap()[:, :])
            wds_t = [cpool.tile([F, 2 * FS], F32, tag=f"wds{li}", name=f"wds_t{li}") for li in range(3)]
            we_t = [cpool.tile([P, FS], F32, tag=f"we{li}", name=f"we_t{li}") for li in range(3)]
            for li in range(3):
                nc.sync.dma_start(wds_t[li][:], t_wds[li].ap()[:, :])
                nc.sync.dma_start(we_t[li][:], t_we[li].ap()[:, :])

            # per-block rhs table: rows 0:NW acat, rows 92:125 We+bias
            rhs_all = pers.tile([P, B * FS], F32)
            nc.vector.memset(rhs_all[:], 0.0)

            nc.gpsimd.dma_start(h_a.ap()[:, :], t_x.ap()[:, :])

            bufs = [h_a, h_b]
            import os as _os
            _nlayers = int(_os.environ.get("KERNEL_LAYERS", "4"))
            for layer in range(_nlayers):
                li = [0, 1, 1, 2][layer]
                layer = 3 if (_nlayers != 4 and layer == _nlayers - 1) else layer
                h_cur = bufs[layer % 2]
                h_nxt = bufs[(layer + 1) % 2]

                import os as _os
                # ---------- projection phase (per-block PE transpose)
                for k in range(B):
                    hblk = sb.tile([P, F], F32, tag="hblk")
                    nc.sync.dma_start(hblk[:], h_cur.ap()[k * P:(k + 1) * P, :])
                    hT_ps = ps.tile([F, P], F32, tag="sm")
                    nc.tensor.transpose(hT_ps[:], hblk[:], ident_t[:])
                    hTs = sb.tile([F, P], F32, tag="hTs")
                    nc.vector.tensor_copy(hTs[:], hT_ps[:])
                    pj = ps.tile([P, 2 * FS], F32, tag="big")
                    nc.tensor.matmul(pj[:], lhsT=hTs[:],
                                     rhs=wds_t[li][:], start=True, stop=True)
                    nc.vector.tensor_copy(
                        rhs_all[0:NW, k * FS:(k + 1) * FS], pj[0:NW, 0:FS])
                    nc.vector.tensor_copy(
                        rhs_all[ROW_EA:KTOT, k * FS:(k + 1) * FS],
                        we_t[li][ROW_EA:KTOT, :])
                    bco = sb.tile([P, FS], F32, tag="bco")
                    nc.vector.tensor_copy(bco[:], pj[:, FS:2 * FS])
                    nc.sync.dma_start(bc_own.ap()[k * P:(k + 1) * P, :], bco[:])

                if _os.environ.get("SKIP_AG"):
                    nc.gpsimd.dma_start(bc_full.ap()[0:NROWS, :], bc_own.ap()[:, :])
                else:
                    nc.gpsimd.collective_compute(
                        "AllGather", mybir.AluOpType.bypass,
                        replica_groups=[list(range(NC))],
                        ins=[bc_own.ap()[:, :]], outs=[bc_full.ap()[:, :]])

                # ---------- edge phase
                for k in range(B):
                    idxb = blk.tile([P, tpb], I32, tag="idxb")
                    nc.sync.dma_start(idxb[:], t_srci.ap()[k, :, :])
                    drelb = blk.tile([P, tpb], F32, tag="drelb")
                    nc.sync.dma_start(drelb[:], t_drel.ap()[k, :, :])
                    drow = blk.tile([1, tpb * P], F32, tag="drow")
                    nc.sync.dma_start(drow[:], t_drow.ap()[k, :, :])
                    lhs_blk = blk.tile([P, tpb * P], F32, tag="lhsb")
                    nc.sync.dma_start(
                        lhs_blk[ROW_EA:KTOT, :],
                        t_eaT.ap()[0:KE, k * tpb * P:(k + 1) * tpb * P])
                    # onehot_ne rows 0:NW of lhs_blk, built in 384-col chunks
                    nchunk = (tpb * P + 383) // 384
                    for ci in range(nchunk):
                        c0 = ci * 384
                        c1 = min((ci + 1) * 384, tpb * P)
                        dT = ps.tile([P, 384], F32, tag="big")
                        nc.tensor.matmul(dT[:, :c1 - c0], lhsT=ones_t[:],
                                         rhs=drow[:, c0:c1],
                                         start=True, stop=True)
                        nc.vector.tensor_scalar(
                            out=lhs_blk[0:NW, c0:c1], in0=dT[0:NW, :c1 - c0],
                            scalar1=iota32_t[0:NW, :1], scalar2=None,
                            op0=mybir.AluOpType.is_equal)
                    # onehot_en for scatter: one broadcast compare
                    oh_en_blk = blk.tile([P, tpb * P], F32, tag="ohen")
                    nc.vector.tensor_tensor(
                        out=oh_en_blk[:].rearrange("p (t n) -> p t n", n=P),
                        in0=drelb[:].rearrange("p (t a) -> p t a", a=1)
                            .to_broadcast([P, tpb, P]),
                        in1=iota_t[:].rearrange("p (a n) -> p a n", a=1)
                            .to_broadcast([P, tpb, P]),
                        op=mybir.AluOpType.is_equal)
                    pre_blk = blk.tile([P, tpb * FS], F32, tag="pre")
                    agg = psagg.tile([P, F], F32, tag="agg")
                    for t in range(tpb):
                        pre_ps = ps.tile([P, FS], F32, tag="big")
                        nc.tensor.matmul(
                            pre_ps[:], lhsT=lhs_blk[0:KTOT, t * P:(t + 1) * P],
                            rhs=rhs_all[0:KTOT, k * FS:(k + 1) * FS],
                            start=True, stop=True)
                        bg = sb.tile([P, FS], F32, tag="bg")
                        if _os.environ.get("SKIP_GATHER"):
                            nc.sync.dma_start(bg[:], bc_full.ap()[0:P, :])
                        else:
                            nc.gpsimd.indirect_dma_start(
                                out=bg[:], out_offset=None, in_=bc_full.ap()[:, :],
                                in_offset=bass.IndirectOffsetOnAxis(
                                    ap=idxb[:, t:t + 1], axis=0))
                        nc.vector.tensor_tensor(
                            out=pre_blk[:, t * FS:(t + 1) * FS],
                            in0=pre_ps[:], in1=bg[:], op=mybir.AluOpType.add)
                    s1 = blk.tile([P, tpb * F], F32, tag="s1")
                    s2 = blk.tile([P, tpb * F], F32, tag="s2")
                    pre3 = pre_blk[:].rearrange("p (t f) -> p t f", f=FS)
                    nc.scalar.activation(
                        s1[:].rearrange("p (t f) -> p t f", f=F),
                        pre3[:, :, 0:F], mybir.ActivationFunctionType.Sigmoid)
                    ab = blk.tile([P, tpb * F], F32, tag="ab")
                    nc.scalar.activation(
                        ab[:].rearrange("p (t f) -> p t f", f=F),
                        pre3[:, :, F:FS], mybir.ActivationFunctionType.Abs)
                    sab = blk.tile([P, tpb * F], F32, tag="sab")
                    nc.scalar.activation(sab[:], ab[:],
                                         mybir.ActivationFunctionType.Sigmoid)
                    lab = blk.tile([P, tpb * F], F32, tag="lab")
                    nc.scalar.activation(
                        lab[:], sab[:],
                        mybir.ActivationFunctionType.Relu
                        if _os.environ.get("SKIP_LN") else
                        mybir.ActivationFunctionType.Ln)
                    rl = blk.tile([P, tpb * F], F32, tag="rl")
                    nc.scalar.activation(
                        rl[:].rearrange("p (t f) -> p t f", f=F),
                        pre3[:, :, F:FS], mybir.ActivationFunctionType.Relu)
                    nc.vector.tensor_tensor(out=s2[:], in0=rl[:], in1=lab[:],
                                            op=mybir.AluOpType.subtract)
                    msg = blk.tile([P, tpb * F], F32, tag="msg")
                    nc.vector.tensor_tensor(out=msg[:], in0=s1[:], in1=s2[:],
                                            op=mybir.AluOpType.mult)
                    for t in range(tpb):
                        nc.tensor.matmul(
                            agg[:], lhsT=oh_en_blk[:, t * P:(t + 1) * P],
                            rhs=msg[:, t * F:(t + 1) * F],
                            start=(t == 0), stop=(t == tpb - 1))
                    # ---------- evacuate
                    hb = sb.tile([P, F], F32, tag="hb")
                    nc.sync.dma_start(hb[:], h_cur.ap()[k * P:(k + 1) * P, :])
                    tsum = sb.tile([P, F], F32, tag="tsum")
                    nc.vector.tensor_tensor(out=tsum[:], in0=agg[:], in1=hb[:],
                                            op=mybir.AluOpType.add)
                    if layer < 3:
                        hn = sb.tile([P, F], F32, tag="hn")
                        nc.scalar.activation(hn[:], tsum[:],
                                             mybir.ActivationFunctionType.Relu)
                        nc.sync.dma_start(h_nxt.ap()[k * P:(k + 1) * P, :], hn[:])
                    else:
                        mx = sb.tile([P, 1], F32, tag="mx")
                        nc.vector.reduce_max(out=mx[:], in_=tsum[:],
                                             axis=mybir.AxisListType.X)
                        mxn = sb.tile([P, 1], F32, tag="mxn")
                        nc.vector.tensor_scalar_mul(mxn[:], mx[:], -1.0)
                        ex = sb.tile([P, F], F32, tag="ex")
                        nc.scalar.activation(ex[:], tsum[:],
                                             mybir.ActivationFunctionType.Exp,
                                             bias=mxn[:, :1])
                        sm = sb.tile([P, 1], F32, tag="sm")
                        nc.vector.reduce_sum(out=sm[:], in_=ex[:],
                                             axis=mybir.AxisListType.X)
                        rc = sb.tile([P, 1], F32, tag="rc")
                        nc.vector.reciprocal(rc[:], sm[:])
                        so = sb.tile([P, F], F32, tag="so")
                        nc.vector.tensor_scalar(out=so[:], in0=ex[:],
                                                scalar1=rc[:, :1], scalar2=None,
                                                op0=mybir.AluOpType.mult)
                        nc.sync.dma_start(t_out.ap()[k * P:(k + 1) * P, :],
                                          so[:])
    nc.compile()
    return nc


if __name__ == "__main__":
    import reference
    import time
    inputs = {k: np.asarray(v) for k, v in reference.setup_inputs().items()}
    t0 = time.time()
    got = kernel(**inputs)
    print("kernel wall:", time.time() - t0)
    exp = np.asarray(reference.reference(**inputs))
    err = np.abs(got - exp).max() / (np.abs(exp).max() + 1e-9)
    print("Relative error:", err)
